# revision 29
# baseline (speedup 1.0000x reference)
"""Bidirectional Mamba block — Trainium2 Bass/Tile kernel, 8-core data-parallel.

Sharding: batch B=8 -> one sample per NeuronCore, zero collectives.

Per-core layout: activations transposed ([channel, time]); matmuls contract
channels on partitions; the selective scan runs as hardware
`tensor_tensor_scan` along free/time (fp32, 1x — no fast DVE mode exists for
scans).  Everything else on the DVE runs in bf16 to hit the 2x_1p packed
mode: b4 = u*B as one broadcast tensor_tensor, and the y-readout as h*C
followed by an in-place binary-tree reduction over the 16 states.

The depthwise causal conv (k=2) is folded into the in_proj matmul: host
pre-scales two weight copies (W0 = conv_w[:,0]*in_w, W1 = conv_w[:,1]*in_w)
and the kernel accumulates W1@x[t] + W0@x[t-1] in PSUM using a zero-padded
x (xTpad has a leading zero column).  conv_b is zero in this model.  This
makes the P1 projection phase carry-free, so P1 of task k+1 is emitted
interleaved into the scan loop of task k (software pipeline) — the PE/ACT
work of the next projection hides under the DVE-bound scan phase.

ACT table discipline: per task the scalar engine sees
[Exp*16][Ln*16][Exp*256 + Tanh of next P1] -> 2 table loads per task.
"""

import numpy as np
import ml_dtypes

import concourse.bass as bass
import concourse.bacc as bacc
import concourse.mybir as mybir
from concourse import tile
from concourse import bass_utils

AL = mybir.AluOpType
AF = mybir.ActivationFunctionType
F32 = mybir.dt.float32
BF16 = mybir.dt.bfloat16

NCORES = 8
MMF = 512          # max moving free dim per matmul
MMDT = BF16        # matmul input dtype
HDT = BF16         # scan-output h dtype


class Dims:
    def __init__(self, L=1024, D=512, DI=2048, DS=16, DTR=32, DFF=2048, TC=512):
        self.L, self.D, self.DI, self.DS, self.DTR, self.DFF = L, D, DI, DS, DTR, DFF
        self.TC = TC
        self.NTC = L // TC
        self.KD = D // 128
        self.KI = DI // 128
        self.KF = DFF // 128
        assert TC % 2 == 0 and L % TC == 0 and DS == 16 and DTR == 32


DIMS = Dims()


# -------------------------------------------------------------------- builder
def build_program(dm: Dims = DIMS):
    nc = bacc.Bacc("TRN2", target_bir_lowering=False, debug=False)

    L, D, DI, DS, DTR = dm.L, dm.D, dm.DI, dm.DS, dm.DTR
    dram = {}

    def din(name, shape, dt=F32):
        dram[name] = nc.dram_tensor(name, list(shape), dt,
                                    kind="ExternalInput").ap()

    din("xTpad", (D, L + 1), MMDT)
    din("xTrpad", (D, L + 1), MMDT)
    din("ones", (128, 128), MMDT)
    din("sel", (DTR, 2 * DS * 128), MMDT)
    for p in ("m1_", "m2_"):
        din(p + "w0T", (D, DI), MMDT)        # conv_w[:,0] * in_w  (xc half)
        din(p + "w1T", (D, DI), MMDT)        # conv_w[:,1] * in_w
        din(p + "zwT", (D, DI), MMDT)        # in_w z half
        din(p + "xproj_wT", (DI, DTR + 2 * DS), MMDT)   # pre-scaled by 0.5
        din(p + "dt_wT", (DTR, DI), MMDT)
        din(p + "out_wT", (DI, D), MMDT)                # pre-scaled by 0.5
        din(p + "A", (DI, DS))                          # -exp(A_log)
        din(p + "dt_b", (128, dm.KI))
        din(p + "cbh", (128, dm.KI))                    # conv_b * 0.5 (tanh bias)
        din(p + "Dph", (128, dm.KI))                    # Dp * 0.5
    din("ln_g", (128, dm.KD)); din("ln_b", (128, dm.KD))
    din("c1_wT", (D, dm.DFF), MMDT); din("c1_b", (128, dm.KF))
    din("c2_wT", (dm.DFF, D), MMDT); din("c2_b", (128, dm.KD))
    outT = nc.dram_tensor("outT", [D, L], F32, kind="ExternalOutput").ap()

    with tile.TileContext(nc) as tc_:
        _emit(nc, tc_, dram, outT, dm)
    nc.compile()
    return nc


def _emit(nc, tc_, dram, outT, dm):
    from contextlib import ExitStack
    L, D, DI, DS, DTR, DFF, TC, NTC = (dm.L, dm.D, dm.DI, dm.DS, dm.DTR,
                                       dm.DFF, dm.TC, dm.NTC)
    KD, KI, KF = dm.KD, dm.KI, dm.KF
    mm = nc.tensor.matmul
    # (dir, t0, width) tasks; dir0 leads with a narrow chunk so the first
    # scan phase starts early (shrinks the V-idle startup ramp).
    tasks = [(d, c * TC, TC) for d in range(2) for c in range(NTC)]

    with ExitStack() as ctx:
        pers = ctx.enter_context(tc_.tile_pool(name="pers", bufs=1))
        wp = ctx.enter_context(tc_.tile_pool(name="wp", bufs=2))
        psmm = ctx.enter_context(tc_.tile_pool(name="psmm", bufs=2, space="PSUM"))
        psacc = ctx.enter_context(tc_.tile_pool(name="psacc", bufs=1, space="PSUM"))
        dpool = ctx.enter_context(tc_.tile_pool(name="dpool", bufs=1, space="DRAM"))
        sctx = ExitStack()
        ssm = sctx.enter_context(tc_.tile_pool(name="ssmp", bufs=1))
        sp = sctx.enter_context(tc_.tile_pool(name="sp", bufs=2))

        ones_sb = pers.tile([128, 128], MMDT, tag="ones", name="ones")
        nc.sync.dma_start(ones_sb[:], dram["ones"][:])
        eps_sb = pers.tile([128, 1], F32, tag="eps", name="eps")
        nc.vector.memset(eps_sb[:], 1e-5)
        sel_sb = ssm.tile([DTR, 2 * DS * 128], MMDT, tag="sel", name="sel")
        nc.sync.dma_start(sel_sb[:], dram["sel"][:])

        y_scr = [dpool.tile([D, L], F32, tag=f"y_scr{i}", name=f"y_scr{i}")
                 for i in range(2)]
        xc_scr = [dpool.tile([DI, L], MMDT, tag=f"xc_scr{i}", name=f"xc_scr{i}")
                  for i in range(2)]
        sz_scr = [dpool.tile([DI, L], BF16, tag=f"sz_scr{i}", name=f"sz_scr{i}")
                  for i in range(2)]

        # per-direction persistent tiles (both directions loaded upfront)
        A_sb, dtw_sb, xpw_sb, carry, vec = [], [], [], [], []
        for d in range(2):
            p = ("m1_", "m2_")[d]
            A_sb.append(ssm.tile([128, KI * DS], F32, tag=f"A{d}", name=f"A{d}"))
            nc.sync.dma_start(
                A_sb[d][:].rearrange("q (k s) -> q k s", k=KI),
                dram[p + "A"].rearrange("(k q) s -> q k s", q=128))
            dtw_sb.append(ssm.tile([DTR, DI], MMDT, tag=f"dtw{d}", name=f"dtw{d}"))
            nc.sync.dma_start(dtw_sb[d][:], dram[p + "dt_wT"][:])
            xpw_sb.append(ssm.tile([128, KI * (DTR + 2 * DS)], MMDT,
                                    tag=f"xpw{d}", name=f"xpw{d}"))
            nc.sync.dma_start(
                xpw_sb[d][:].rearrange("q (k c) -> q k c", k=KI),
                dram[p + "xproj_wT"].rearrange("(k q) c -> q k c", q=128))
            carry.append(ssm.tile([128, KI * DS], F32, tag=f"carry{d}",
                                   name=f"carry{d}"))
            v = {}
            for nm in ("dt_b", "cbh", "Dph"):
                v[nm] = ssm.tile([128, KI], F32, tag=f"{nm}{d}", name=f"{nm}{d}")
                nc.sync.dma_start(v[nm][:], dram[p + nm][:])
            vec.append(v)

        # shared scan-phase tiles (one task at a time)
        bcB = ssm.tile([128, DS * TC], BF16, tag="bcB", name="bcB")
        bcC = ssm.tile([128, DS * TC], BF16, tag="bcC", name="bcC")
        dt_h = ssm.tile([128, KI * TC], BF16, tag="dt_h", name="dt_h")
        # acat: segmented scan multiplier buffer [DS, TC+1] per segment; the
        # first column of each segment is an injected carry slot (a=0 there).
        # Its storage doubles as the softplus Exp scratch (t1) at chunk start.
        acat = ssm.tile([128, DS * (TC + 1)], F32, tag="acat", name="acat")
        t1 = ssm.tile([128, KI * TC], BF16, tag="t1", name="t1")[:]
        b4c = ssm.tile([128, DS * (TC + 1)], BF16, tag="b4c", name="b4c")
        hc = ssm.tile([128, DS * (TC + 1)], HDT, tag="hc", name="hc")
        dbc_sb = {t: ssm.tile([64, TC], MMDT, tag=f"dbc{t}", name=f"dbc{t}")
                  for t in tasks}
        dbc32 = {t: ssm.tile([32, TC], MMDT, tag=f"dbc32{t}", name=f"dbc32{t}")
                 for t in tasks}
        dbc_ps = {}

        # ------------------------------------------------------- P1 pieces
        def p1_load_x(task):
            d, t0, tcc = task
            xnm = ("xTpad", "xTrpad")[d]
            xtc = [sp.tile([128, TC + 1], MMDT, tag=f"xtc{k}", name=f"xtc{k}")
                   for k in range(KD)]
            for k in range(KD):
                nc.sync.dma_start(
                    xtc[k][:, 0:tcc + 1],
                    dram[xnm][k * 128:(k + 1) * 128, t0:t0 + tcc + 1])
            return xtc

        def p1_piece(task, kt, xtc):
            """One kt-group of the projection phase: xc (kt<KI) or z."""
            d, t0, tcc = task
            p = ("m1_", "m2_")[d]
            if kt < KI:         # conv half
                ps = psmm.tile([128, TC], F32, tag="mm", name="mm")
                w1 = wp.tile([128, KD * 128], MMDT, tag="w_in", name="w_in")
                nc.sync.dma_start(
                    w1[:].rearrange("q (k e) -> q k e", k=KD),
                    dram[p + "w1T"].rearrange("(k q) e -> q k e", q=128)
                    [:, :, kt * 128:(kt + 1) * 128])
                w0 = wp.tile([128, KD * 128], MMDT, tag="w_in", name="w_in")
                nc.sync.dma_start(
                    w0[:].rearrange("q (k e) -> q k e", k=KD),
                    dram[p + "w0T"].rearrange("(k q) e -> q k e", q=128)
                    [:, :, kt * 128:(kt + 1) * 128])
                for k in range(KD):
                    mm(ps[:, 0:tcc], w1[:, k * 128:(k + 1) * 128],
                       xtc[k][:, 1:tcc + 1], start=(k == 0), stop=False)
                for k in range(KD):
                    mm(ps[:, 0:tcc], w0[:, k * 128:(k + 1) * 128],
                       xtc[k][:, 0:tcc], start=False, stop=(k == KD - 1))
                th = sp.tile([128, TC], F32, tag="th", name="th")
                nc.scalar.activation(th[:, 0:tcc], ps[:, 0:tcc], AF.Tanh,
                                     bias=vec[d]["cbh"][:, kt:kt + 1], scale=0.5)
                xck = sp.tile([128, TC], MMDT, tag="xck", name="xck")
                # xc = (1+tanh(conv/2)) * conv = 2*silu(conv)   (conv_b=0)
                nc.vector.scalar_tensor_tensor(xck[:, 0:tcc], th[:, 0:tcc],
                                               1.0, ps[:, 0:tcc],
                                               AL.add, AL.mult)
                nc.sync.dma_start(
                    xc_scr[d][kt * 128:(kt + 1) * 128, t0:t0 + tcc],
                    xck[:, 0:tcc])
                if kt == 0:
                    dbc_ps[task] = psacc.tile([64, TC], F32, tag="acc_dbc",
                                              name="acc_dbc")
                mm(dbc_ps[task][:, 0:tcc], xpw_sb[d][:, kt * 64:(kt + 1) * 64],
                   xck[:, 0:tcc], start=(kt == 0), stop=(kt == KI - 1))
                if kt == KI - 1:
                    nc.scalar.copy(dbc_sb[task][:, 0:tcc],
                                   dbc_ps[task][:, 0:tcc])
                    nc.scalar.copy(dbc32[task][:, 0:tcc],
                                   dbc_sb[task][DTR:DTR + 2 * DS, 0:tcc])
            else:               # z half
                kz = kt - KI
                zps = psmm.tile([128, TC], F32, tag="mm", name="mm")
                wz = wp.tile([128, KD * 128], MMDT, tag="w_in", name="w_in")
                nc.sync.dma_start(
                    wz[:].rearrange("q (k e) -> q k e", k=KD),
                    dram[p + "zwT"].rearrange("(k q) e -> q k e", q=128)
                    [:, :, kz * 128:(kz + 1) * 128])
                for k in range(KD):
                    mm(zps[:, 0:tcc], wz[:, k * 128:(k + 1) * 128],
                       xtc[k][:, 1:tcc + 1], start=(k == 0), stop=(k == KD - 1))
                th = sp.tile([128, TC], F32, tag="th", name="th")
                nc.scalar.activation(th[:, 0:tcc], zps[:, 0:tcc], AF.Tanh,
                                     scale=0.5)
                szk = sp.tile([128, TC], BF16, tag="szk", name="szk")
                nc.vector.scalar_tensor_tensor(szk[:, 0:tcc], th[:, 0:tcc],
                                               1.0, zps[:, 0:tcc],
                                               AL.add, AL.mult)
                nc.sync.dma_start(
                    sz_scr[d][kz * 128:(kz + 1) * 128, t0:t0 + tcc],
                    szk[:, 0:tcc])

        # ------------------------------------------------------------ P2
        exp_pre = set()

        def dt_exp_one(task, kt):
            # dps matmul + Exp for one kt of softplus (writes t1)
            d, t0, tcc = task
            dps = psmm.tile([128, TC], F32, tag="mm", name="mm")
            mm(dps[:, 0:tcc], dtw_sb[d][:, kt * 128:(kt + 1) * 128],
               dbc_sb[task][0:DTR, 0:tcc], start=True, stop=True)
            nc.scalar.activation(t1[:, kt * tcc:(kt + 1) * tcc],
                                 dps[:, 0:tcc], AF.Exp,
                                 bias=vec[d]["dt_b"][:, kt:kt + 1])

        def dt_ln(task, kt_lo, kt_hi):
            # one fused Ln over [kt_lo, kt_hi) segments: a single ACTIVATE
            # cannot be interleaved by the scheduler, so the Exp/Ln table
            # sets swap once per task instead of per kt.
            d, t0, tcc = task
            nc.scalar.activation(dt_h[:, kt_lo * tcc:kt_hi * tcc],
                                 t1[:, kt_lo * tcc:kt_hi * tcc],
                                 AF.Ln, bias=1.0)

        def p2(task, next_task, self_x=None, first=False):
            d, t0, tcc = task
            p = ("m1_", "m2_")[d]
            seg = DS * (tcc + 1)
            # dt = softplus via Exp then Ln, each batched (table sets differ);
            # the Exp half may have been pre-emitted by the previous task.
            # For the very first task, only kt0's softplus and the B-half of
            # the broadcast go ahead of the first scan; the rest is emitted
            # under it (shortens the cold-start critical path).
            nkt0 = 1 if first else KI
            if task not in exp_pre:
                for kt in range(nkt0):
                    dt_exp_one(task, kt)
            dt_ln(task, 0, nkt0)
            nc.vector.memset(acat[:, 0:seg][:, 0::tcc + 1], 0.0)
            if t0 == 0:
                nc.vector.memset(carry[d][:], 0.0)
            # B/C broadcast via selection matmuls
            def sel_bcast(s):
                bps = psmm.tile([128, TC], F32, tag="mm", name="mm")
                mm(bps[:, 0:tcc], sel_sb[:, s * 128:(s + 1) * 128],
                   dbc32[task][:, 0:tcc], start=True, stop=True)
                if s < DS:   # B broadcast, pre-halved (folds u's 0.5 factor)
                    nc.scalar.activation(bcB[:, s * tcc:(s + 1) * tcc],
                                         bps[:, 0:tcc], AF.Identity, scale=0.5)
                else:
                    nc.scalar.copy(bcC[:, (s - DS) * tcc:(s - DS + 1) * tcc],
                                   bps[:, 0:tcc])

            for s in range(DS if first else 2 * DS):
                sel_bcast(s)

            nxt_x = p1_load_x(next_task) if next_task is not None else None

            y_ps = [psacc.tile([128, TC], F32, tag=f"acc{k}", name=f"acc{k}")
                    for k in range(KD)]
            for kt in range(KI):
                if self_x is not None:
                    p1_piece(task, KI + kt, self_x)
                dts = dt_h[:, kt * tcc:(kt + 1) * tcc]
                xck = sp.tile([128, TC], MMDT, tag="xck2", name="xck2")
                nc.sync.dma_start(
                    xck[:, 0:tcc],
                    xc_scr[d][kt * 128:(kt + 1) * 128, t0:t0 + tcc])
                u = sp.tile([128, TC], BF16, tag="u", name="u")
                nc.vector.tensor_tensor(u[:, 0:tcc], xck[:, 0:tcc], dts,
                                        AL.mult)
                # carry slots: b = carry, a = 0 (memset above) -> state reload
                nc.scalar.copy(b4c[:, 0:seg][:, 0::tcc + 1],
                               carry[d][:, kt * DS:(kt + 1) * DS])
                nc.vector.tensor_tensor(
                    b4c[:, 0:seg].rearrange("q (s t) -> q s t", t=tcc + 1)
                    [:, :, 1:],
                    u[:, 0:tcc].rearrange("q (o t) -> q o t", o=1)
                    .to_broadcast((128, DS, tcc)),
                    bcB[:, 0:DS * tcc].rearrange("q (s t) -> q s t", s=DS),
                    AL.mult)
                for s in range(DS):
                    nc.scalar.activation(
                        acat[:, s * (tcc + 1) + 1:(s + 1) * (tcc + 1)], dts,
                        AF.Exp, scale=A_sb[d][:, kt * DS + s:kt * DS + s + 1])
                nc.vector.tensor_tensor_scan(hc[:, 0:seg], acat[:, 0:seg],
                                             b4c[:, 0:seg], 0.0,
                                             AL.mult, AL.add)
                if first and kt == 0:
                    for s in range(DS, 2 * DS):
                        sel_bcast(s)
                    for kt2 in range(1, KI):
                        dt_exp_one(task, kt2)
                    dt_ln(task, 1, KI)
                if t0 + tcc != L:
                    nc.scalar.copy(carry[d][:, kt * DS:(kt + 1) * DS],
                                   hc[:, 0:seg][:, tcc::tcc + 1])
                hseg = hc[:, 0:seg].rearrange("q (s t) -> q s t",
                                              t=tcc + 1)[:, :, 1:]
                nc.vector.tensor_tensor(
                    hseg, hseg,
                    bcC[:, 0:DS * tcc].rearrange("q (s t) -> q s t", s=DS),
                    AL.mult)
                ns = DS
                while ns > 1:
                    ns //= 2
                    hv = hc[:, 0:seg].rearrange("q (s t) -> q s t", t=tcc + 1)
                    nc.vector.tensor_tensor(
                        hv[:, 0:ns, 1:], hv[:, 0:ns, 1:],
                        hv[:, ns:2 * ns, 1:], AL.add)
                szk = sp.tile([128, TC], BF16, tag="szk2", name="szk2")
                nc.sync.dma_start(
                    szk[:, 0:tcc],
                    sz_scr[d][kt * 128:(kt + 1) * 128, t0:t0 + tcc])
                q = sp.tile([128, TC], BF16, tag="qd", name="qd")
                nc.scalar.activation(q[:, 0:tcc], xck[:, 0:tcc], AF.Identity,
                                     scale=vec[d]["Dph"][:, kt:kt + 1])
                yv = sp.tile([128, TC], BF16, tag="yv", name="yv")
                nc.vector.tensor_tensor(yv[:, 0:tcc], q[:, 0:tcc],
                                        hc[:, 1:tcc + 1], AL.add)
                g = sp.tile([128, TC], MMDT, tag="g", name="g")
                nc.vector.tensor_tensor(g[:, 0:tcc], yv[:, 0:tcc],
                                        szk[:, 0:tcc], AL.mult)
                w4 = wp.tile([128, KD * 128], MMDT, tag="w_out", name="w_out")
                nc.sync.dma_start(
                    w4[:], dram[p + "out_wT"][kt * 128:(kt + 1) * 128, :])
                for k in range(KD):
                    mm(y_ps[k][:, 0:tcc], w4[:, k * 128:(k + 1) * 128],
                       g[:, 0:tcc], start=(kt == 0), stop=(kt == KI - 1))
                # software pipeline: 2 projection pieces of the next task
                if next_task is not None:
                    for j in (2 * kt, 2 * kt + 1):
                        p1_piece(next_task, j, nxt_x)
                    if KI == 16 and kt >= 10:
                        for j in range((kt - 10) * KI // 6,
                                       (kt - 9) * KI // 6):
                            dt_exp_one(next_task, j)
                        exp_pre.add(next_task)
            for k in range(KD):
                yo = sp.tile([128, TC], F32, tag="yo", name="yo")
                nc.scalar.copy(yo[:, 0:tcc], y_ps[k][:, 0:tcc])
                nc.sync.dma_start(
                    y_scr[d][k * 128:(k + 1) * 128, t0:t0 + tcc],
                    yo[:, 0:tcc])

        # ---------------------------------------------------------- schedule
        first_x = p1_load_x(tasks[0])
        for kt in range(KI):
            p1_piece(tasks[0], kt, first_x)
        for ti, task in enumerate(tasks):
            p2(task, tasks[ti + 1] if ti + 1 < len(tasks) else None,
               self_x=first_x if ti == 0 else None, first=(ti == 0))

        sctx.close()

        # ============================================================ phase C
        with tc_.tile_pool(name="cpool", bufs=1) as cp, \
             tc_.tile_pool(name="csp", bufs=2) as csp:
            ln_g = cp.tile([128, KD], F32, tag="ln_g", name="ln_g")
            ln_b = cp.tile([128, KD], F32, tag="ln_b", name="ln_b")
            c1b = cp.tile([128, KF], F32, tag="c1b", name="c1b")
            c2b = cp.tile([128, KD], F32, tag="c2b", name="c2b")
            for nm, t in (("ln_g", ln_g), ("ln_b", ln_b), ("c1_b", c1b),
                          ("c2_b", c2b)):
                nc.sync.dma_start(t[:], dram[nm][:])
            CH = min(MMF, L)

            def layernorm(in_tiles, out_tiles, cix, lnix):
                """Column-wise layernorm of one CH-column chunk."""
                sums = csp.tile([1, CH], MMDT, tag="ln_srow", name="ln_srow")
                sqs = csp.tile([1, CH], MMDT, tag="ln_qrow", name="ln_qrow")
                sps = psacc.tile([1, CH], F32, tag="mmrow", name="mmrow")
                for k in range(KD):
                    mm(sps[:], ones_sb[:, 0:1], in_tiles[k][:],
                       start=(k == 0), stop=(k == KD - 1))
                nc.scalar.copy(sums[:], sps[:])
                qps = psacc.tile([1, CH], F32, tag="mmrow", name="mmrow")
                for k in range(KD):
                    sq = csp.tile([128, CH], MMDT, tag="ln_sq", name="ln_sq")
                    nc.vector.tensor_tensor(sq[:], in_tiles[k][:],
                                            in_tiles[k][:], AL.mult)
                    mm(qps[:], ones_sb[:, 0:1], sq[:],
                       start=(k == 0), stop=(k == KD - 1))
                nc.scalar.copy(sqs[:], qps[:])
                mu = csp.tile([128, CH], F32, tag="ln_mu", name="ln_mu")
                inv = csp.tile([128, CH], F32, tag="ln_inv", name="ln_inv")
                mps = psmm.tile([128, CH], F32, tag="mm", name="mm")
                mm(mps[:], ones_sb[0:1, :], sums[:], start=True, stop=True)
                nc.vector.tensor_scalar(mu[:], mps[:], 1.0 / D, None, AL.mult)
                qrep = psmm.tile([128, CH], F32, tag="mm", name="mm")
                mm(qrep[:], ones_sb[0:1, :], sqs[:], start=True, stop=True)
                ex2 = csp.tile([128, CH], F32, tag="ln_ex2", name="ln_ex2")
                nc.vector.tensor_scalar(ex2[:], qrep[:], 1.0 / D, None, AL.mult)
                var = csp.tile([128, CH], F32, tag="ln_var", name="ln_var")
                nc.vector.tensor_tensor(var[:], mu[:], mu[:], AL.mult)
                nc.vector.tensor_tensor(var[:], ex2[:], var[:], AL.subtract)
                sd = csp.tile([128, CH], F32, tag="ln_sd", name="ln_sd")
                nc.scalar.activation(sd[:], var[:], AF.Sqrt, bias=eps_sb[:])
                nc.vector.reciprocal(inv[:], sd[:])
                for k in range(KD):
                    xm = csp.tile([128, CH], F32, tag="ln_xm", name="ln_xm")
                    nc.vector.tensor_tensor(xm[:], in_tiles[k][:], mu[:],
                                            AL.subtract)
                    nc.vector.tensor_tensor(xm[:], xm[:], inv[:], AL.mult)
                    nc.vector.tensor_scalar(out_tiles[k][:], xm[:],
                                            ln_g[:, k:k + 1], ln_b[:, k:k + 1],
                                            AL.mult, AL.add)

            NFH = min(8, KF)
            NCC = L // CH

            def phc_part1(cix, nk):
                y3p = [csp.tile([128, CH], MMDT, tag=f"y3p{k}", name=f"y3p{k}",
                                bufs=NCC) for k in range(KD)]
                for k in range(KD):
                    xt = csp.tile([128, CH], MMDT, tag="c_x", name="c_x")
                    y1t = csp.tile([128, CH], F32, tag="c_y1", name="c_y1")
                    y2t = csp.tile([128, CH], F32, tag="c_y2", name="c_y2")
                    nc.sync.dma_start(
                        xt[:],
                        dram["xTpad"][k * 128:(k + 1) * 128, 1 + nk:1 + nk + CH])
                    nc.sync.dma_start(y1t[:],
                                      y_scr[0][k * 128:(k + 1) * 128, nk:nk + CH])
                    nc.sync.dma_start(
                        y2t[:],
                        y_scr[1][k * 128:(k + 1) * 128, L - nk - CH:L - nk])
                    t12 = csp.tile([128, CH], F32, tag="c_t12", name="c_t12")
                    nc.vector.tensor_tensor(t12[:], y1t[:], y2t[:, ::-1], AL.add)
                    nc.vector.tensor_tensor(y3p[k][:], xt[:], t12[:], AL.add)
                y3 = [csp.tile([128, CH], MMDT, tag=f"y3_{k}", name=f"y3_{k}",
                               bufs=NCC) for k in range(KD)]
                layernorm(y3p, y3, cix, 0)
                return y3p, y3

            def phc_part2(cix, nk, y3p, y3):
                ypre = y3p
                yacc = [psacc.tile([128, CH], F32, tag=f"acc{k}", name=f"acc{k}")
                        for k in range(KD)]
                for fh in range(KF // NFH):
                    hbuf = []
                    for f2 in range(NFH):
                        f = fh * NFH + f2
                        hps = psmm.tile([128, CH], F32, tag="mm", name="mm")
                        wc1 = wp.tile([128, KD * 128], MMDT, tag="w_c1",
                                      name="w_c1")
                        nc.sync.dma_start(
                            wc1[:].rearrange("q (k e) -> q k e", k=KD),
                            dram["c1_wT"].rearrange("(k q) e -> q k e", q=128)
                            [:, :, f * 128:(f + 1) * 128])
                        for k in range(KD):
                            mm(hps[:], wc1[:, k * 128:(k + 1) * 128],
                               y3[k][:], start=(k == 0), stop=(k == KD - 1))
                        hb = csp.tile([128, CH], MMDT, tag=f"hb{f2}",
                                      name=f"hb{f2}", bufs=1)
                        nc.scalar.activation(hb[:], hps[:], AF.Relu,
                                             bias=c1b[:, f:f + 1])
                        hbuf.append(hb)
                    for f2 in range(NFH):
                        f = fh * NFH + f2
                        wc2 = wp.tile([128, KD * 128], MMDT, tag="w_c2",
                                      name="w_c2")
                        nc.sync.dma_start(
                            wc2[:], dram["c2_wT"][f * 128:(f + 1) * 128, :])
                        for k in range(KD):
                            mm(yacc[k][:], wc2[:, k * 128:(k + 1) * 128],
                               hbuf[f2][:], start=(f == 0), stop=(f == KF - 1))
                for k in range(KD):
                    nc.vector.scalar_tensor_tensor(
                        ypre[k][:], yacc[k][:], c2b[:, k:k + 1],
                        y3[k][:], AL.add, AL.add)
                outs = [csp.tile([128, CH], F32, tag=f"out{k}", name=f"out{k}")
                        for k in range(KD)]
                layernorm(ypre, outs, cix, 1)
                for k in range(KD):
                    nc.sync.dma_start(outT[k * 128:(k + 1) * 128, nk:nk + CH],
                                      outs[k][:])

            staged = [phc_part1(cix, nk)
                      for cix, nk in enumerate(range(0, L, CH))]
            for cix, nk in enumerate(range(0, L, CH)):
                phc_part2(cix, nk, *staged[cix])


# ------------------------------------------------------------------ host side
_PROG_CACHE = {}


def _get_prog():
    if "full" not in _PROG_CACHE:
        _PROG_CACHE["full"] = build_program(DIMS)
    return _PROG_CACHE["full"]


def host_prep(inputs, dm: Dims = DIMS):
    f = np.float32
    bf = ml_dtypes.bfloat16
    x = np.asarray(inputs["x"], dtype=f)
    KI, KD, KF = dm.KI, dm.KD, dm.KF
    D, L, DI = dm.D, dm.L, dm.DI

    def vt(v, n):
        return np.ascontiguousarray(np.asarray(v, f).reshape(n, 128).T)

    c = {}
    sel = np.zeros((dm.DTR, 2 * dm.DS * 128), f)
    for s in range(2 * dm.DS):
        sel[s, s * 128:(s + 1) * 128] = 1.0
    c["sel"] = sel.astype(bf)
    c["ones"] = np.ones((128, 128), bf)
    for p in ("m1_", "m2_"):
        in_w = np.asarray(inputs[p + "in_w"], f)          # (2DI, D)
        cw = np.asarray(inputs[p + "conv_w"], f)          # (DI, 2)
        c[p + "w0T"] = np.ascontiguousarray(
            (in_w[:DI] * cw[:, 0:1]).T).astype(bf)
        c[p + "w1T"] = np.ascontiguousarray(
            (in_w[:DI] * cw[:, 1:2]).T).astype(bf)
        c[p + "zwT"] = np.ascontiguousarray(in_w[DI:].T).astype(bf)
        c[p + "xproj_wT"] = np.ascontiguousarray(
            0.5 * np.asarray(inputs[p + "xproj_w"], f).T).astype(bf)
        c[p + "dt_wT"] = np.ascontiguousarray(
            np.asarray(inputs[p + "dt_w"], f).T).astype(bf)
        c[p + "out_wT"] = np.ascontiguousarray(
            0.5 * np.asarray(inputs[p + "out_w"], f).T).astype(bf)
        c[p + "A"] = np.ascontiguousarray(-np.exp(np.asarray(inputs[p + "A_log"], f)))
        c[p + "dt_b"] = vt(inputs[p + "dt_b"], KI)
        c[p + "cbh"] = vt(0.5 * np.asarray(inputs[p + "conv_b"], f), KI)
        c[p + "Dph"] = vt(0.5 * np.asarray(inputs[p + "Dp"], f), KI)
    c["ln_g"] = vt(inputs["ln_g"], KD)
    c["ln_b"] = vt(inputs["ln_b"], KD)
    c["c1_wT"] = np.ascontiguousarray(np.asarray(inputs["c1_w"], f).T).astype(bf)
    c["c1_b"] = vt(inputs["c1_b"], KF)
    c["c2_wT"] = np.ascontiguousarray(np.asarray(inputs["c2_w"], f).T).astype(bf)
    c["c2_b"] = vt(inputs["c2_b"], KD)

    in_maps = []
    for b in range(x.shape[0]):
        m = dict(c)
        xp = np.zeros((D, L + 1), f)
        xp[:, 1:] = x[b].T
        m["xTpad"] = xp.astype(bf)
        xr = np.zeros((D, L + 1), f)
        xr[:, 1:] = x[b][::-1].T
        m["xTrpad"] = xr.astype(bf)
        in_maps.append(m)
    return in_maps


def kernel(**inputs):
    nc = _get_prog()
    in_maps = host_prep(inputs)
    res = bass_utils.run_bass_kernel_spmd(nc, in_maps, core_ids=list(range(NCORES)))
    return np.stack([np.ascontiguousarray(o["outT"].T) for o in res.results], axis=0)
